# revision 16
# baseline (speedup 1.0000x reference)
"""DecorrLinear fused kernel for one TRN2 chip (8 NeuronCores), via Bass/Tile.

Computes, for x:(B,L,D) f32, W:(D,D), bias:(D,), decorr:(D,D), sample_idx:(B,NS):
  y    = x @ (W @ decorr)^T + bias
  xs   = gather(x, sample_idx) @ decorr^T            # (N, D), N = B*NS
  corr = (sum_n r_n^2 - sum X2^2) / (N*D*D)          # r_n = rowsum(X2), X2 = xs^2
  whit = (sum X2^2 - 2*sum X2 + N*D) / (N*D*D)
  grad = 0.5*(xs^T xs / N) - 0.5*I                   # KAPPA = 0.5

Sharding: data-parallel over tokens (y path) and over sampled rows (loss path).
WdT = (W@decorr)^T is computed sharded (each core one 128-row strip) and
AllGathered; G = xs^T xs partials are ReduceScattered so each core finalizes its
own 128-row strip of grad; the three scalar partial sums are AllReduced.

Matmuls run as float32r (fp32 storage, ~tf32 precision, full PE rate at N>=256).
"""

from contextlib import ExitStack

import numpy as np

import concourse.bacc as bacc
import concourse.bass as bass
import concourse.tile as tile
from concourse import mybir
from concourse.bass_utils import run_bass_kernel_spmd

# Problem shape (hardcoded per contract)
B, L, D = 4, 8192, 1024
NS = 2048                      # n_samples per batch row
N_ROWS = B * NS                # 8192 sampled rows total
NCORES = 8
TOK = B * L                    # 32768 tokens total
TOK_C = TOK // NCORES          # 4096 tokens per core
ROWS_C = N_ROWS // NCORES      # 1024 sampled rows per core
KAPPA = 0.5
DENOM = float(N_ROWS) * D * D

F32 = mybir.dt.float32
F32R = mybir.dt.float32r

P = 128                        # partitions
KT = D // P                    # 8 contraction tiles
FD = 512                       # matmul moving free dim
OC = D // FD                   # 2 output column chunks
SC_TOK = 256                   # token superchunk (DMA slab width)
N_SC = TOK_C // SC_TOK         # 8 superchunks
T_PER_SC = SC_TOK // P         # 4 token tiles per superchunk
RG = [list(range(NCORES))]


def _build():
    nc = bacc.Bacc("TRN2", target_bir_lowering=False, debug=False, num_devices=NCORES)

    xT = nc.dram_tensor("xT", [D, TOK_C], F32R, kind="ExternalInput")
    selT = nc.dram_tensor("selT", [D, ROWS_C], F32R, kind="ExternalInput")
    wT = nc.dram_tensor("wT", [D, D], F32R, kind="ExternalInput")
    dec_strip = nc.dram_tensor("dec_strip", [D, P], F32R, kind="ExternalInput")
    decorrT = nc.dram_tensor("decorrT", [D, D], F32R, kind="ExternalInput")
    bias_b = nc.dram_tensor("bias_b", [P, D], F32, kind="ExternalInput")
    halfeye = nc.dram_tensor("halfeye", [P, D], F32, kind="ExternalInput")

    y_out = nc.dram_tensor("y", [TOK_C, D], F32, kind="ExternalOutput")
    grad_out = nc.dram_tensor("grad", [P, D], F32, kind="ExternalOutput")
    losses_out = nc.dram_tensor("losses", [1, 8], F32, kind="ExternalOutput")

    with tile.TileContext(nc) as tc:
        with (
            tc.tile_pool(name="res", bufs=1) as res,          # long-lived SBUF
            tc.tile_pool(name="psum", bufs=6, space="PSUM") as psum_pool,
            tc.tile_pool(name="psum_sc", bufs=1, space="PSUM") as psum_sc_pool,
            tc.tile_pool(name="dram", bufs=1, space="DRAM") as dram,
            tc.tile_pool(name="evict", bufs=3) as evict_pool, # PSUM->SBUF staging
            tc.tile_pool(name="scratch", bufs=1) as scratch,  # stats scratch
            tc.tile_pool(name="xt", bufs=4) as xt_pool,       # y-path slabs
            tc.tile_pool(name="yev", bufs=7) as y_pool,
            tc.tile_pool(name="dctp", bufs=1) as dct_pool,    # decorrT resident
            tc.tile_pool(name="sslabp", bufs=3) as sslab_pool,
        ):
            # ---- long-lived tiles -------------------------------------------
            wdT_sb = [res.tile([P, D], F32R, name=f"wdT{i}") for i in range(KT)]
            bias_sb = res.tile([P, D], F32, name="bias_sb")
            he_sb = res.tile([P, D], F32, name="he_sb")
            acc_A = res.tile([P, 1], F32, name="acc_A")   # sum r_n^2
            acc_B = res.tile([P, 1], F32, name="acc_B")   # sum X2^2
            acc_C = res.tile([P, 1], F32, name="acc_C")   # sum X2
            ones = res.tile([P, 1], F32, name="ones")
            sc_pack = res.tile([P, 4], F32, name="sc_pack")

            nc.vector.memset(acc_A[:], 0.0)
            nc.vector.memset(acc_B[:], 0.0)
            nc.vector.memset(acc_C[:], 0.0)
            nc.vector.memset(ones[:], 1.0)
            nc.vector.memset(sc_pack[:], 0.0)

            g_part = dram.tile([D, D], F32, name="g_part")
            g_red = dram.tile([P, D], F32, name="g_red")
            sc_part = dram.tile([1, 8], F32, name="sc_part")
            sc_red = dram.tile([1, 8], F32, name="sc_red", addr_space="Shared")
            wd_strip_d = dram.tile([P, D], F32, name="wd_strip_d")
            wd_full = dram.tile([D, D], F32, name="wd_full", addr_space="Shared")

            dct_sb = [dct_pool.tile([P, D], F32R, name=f"dct{k}") for k in range(KT)]
            for k in range(KT):
                nc.sync.dma_start(out=dct_sb[k][:], in_=decorrT[k * P:(k + 1) * P, :])

            # ---- phase 1: own 128-row strip of WdT = decorr^T @ W^T ---------
            # core c computes WdT[128c:128c+128, :] = dec_strip^T @ W^T
            with tc.tile_pool(name="w1", bufs=1) as w1:
                ds_sb = [w1.tile([P, P], F32R, name=f"ds{k}") for k in range(KT)]
                wt_sb = [w1.tile([P, D], F32R, name=f"wt{k}") for k in range(KT)]
                for k in range(KT):
                    nc.sync.dma_start(out=ds_sb[k][:], in_=dec_strip[k * P:(k + 1) * P, :])
                    nc.sync.dma_start(out=wt_sb[k][:], in_=wT[k * P:(k + 1) * P, :])
                wds_sb = w1.tile([P, D], F32, name="wds_sb")
                for jc in range(OC):
                    ps = psum_pool.tile([P, FD], F32, name="mm_ps", tag="mm")
                    for k in range(KT):
                        nc.tensor.matmul(
                            ps[:],
                            ds_sb[k][:],
                            wt_sb[k][:, jc * FD:(jc + 1) * FD],
                            start=(k == 0),
                            stop=(k == KT - 1),
                        )
                    nc.vector.tensor_copy(wds_sb[:, jc * FD:(jc + 1) * FD], ps[:])
                nc.scalar.dma_start(out=wd_strip_d[:], in_=wds_sb[:])

            nc.gpsimd.collective_compute(
                "AllGather",
                mybir.AluOpType.bypass,
                replica_groups=RG,
                ins=[wd_strip_d[:]],
                outs=[wd_full[:]],
            )
            for k in range(KT):
                nc.gpsimd.dma_start(
                    out=wdT_sb[k][:], in_=wd_full[k * P:(k + 1) * P, :].bitcast(F32R)
                )

            # ---- phase 2: xs = selT^T @ decorr^T + stats --------------------
            with tc.tile_pool(name="w2", bufs=1) as w2:
                xs_sb = [w2.tile([P, D], F32R, name=f"xs{i}") for i in range(KT)]
                selT_r = selT.rearrange("(kt p) r -> p kt r", p=P)
                for rt in range(KT):
                    sslab = sslab_pool.tile([P, KT, P], F32R, name="sslab", tag="sslab")
                    nc.sync.dma_start(
                        out=sslab[:], in_=selT_r[:, :, rt * P:(rt + 1) * P]
                    )
                    for jc in range(OC):
                        ps = psum_pool.tile([P, FD], F32, name="mm_ps", tag="mm")
                        for k in range(KT):
                            nc.tensor.matmul(
                                ps[:],
                                sslab[:, k, :],
                                dct_sb[k][:, jc * FD:(jc + 1) * FD],
                                start=(k == 0),
                                stop=(k == KT - 1),
                            )
                        nc.vector.tensor_copy(xs_sb[rt][:, jc * FD:(jc + 1) * FD], ps[:])

                    # per-row-tile stats on X2 = xs^2
                    xsf = xs_sb[rt][:].bitcast(F32)
                    x2 = scratch.tile([P, D], F32, name="x2", tag="x2")
                    rsum = scratch.tile([P, 1], F32, name="rsum", tag="rsum")
                    tvec = scratch.tile([P, 1], F32, name="tvec", tag="tvec")
                    nc.vector.tensor_mul(x2[:], xsf, xsf)
                    nc.vector.tensor_reduce(
                        rsum[:], x2[:], axis=mybir.AxisListType.X, op=mybir.AluOpType.add
                    )
                    nc.vector.tensor_mul(x2[:], x2[:], x2[:])
                    nc.vector.tensor_reduce(
                        tvec[:], x2[:], axis=mybir.AxisListType.X,
                        op=mybir.AluOpType.add,
                    )
                    nc.vector.scalar_tensor_tensor(
                        out=acc_A[:],
                        in0=rsum[:],
                        scalar=rsum[:, 0:1],
                        in1=acc_A[:],
                        op0=mybir.AluOpType.mult,
                        op1=mybir.AluOpType.add,
                    )
                    nc.vector.tensor_add(acc_C[:], acc_C[:], rsum[:])
                    nc.vector.tensor_add(acc_B[:], acc_B[:], tvec[:])

                # ---- phase 3: G = xs^T @ xs -> DRAM partial -----------------
                for d1 in range(KT):
                    for jc in range(OC):
                        ps = psum_pool.tile([P, FD], F32, name="mm_ps", tag="mm")
                        for rt in range(KT):
                            nc.tensor.matmul(
                                ps[:],
                                xs_sb[rt][:, d1 * P:(d1 + 1) * P],
                                xs_sb[rt][:, jc * FD:(jc + 1) * FD],
                                start=(rt == 0),
                                stop=(rt == KT - 1),
                            )
                        gev = evict_pool.tile([P, FD], F32, name="gev", tag="gev")
                        nc.vector.tensor_copy(gev[:], ps[:])
                        nc.scalar.dma_start(
                            out=g_part[d1 * P:(d1 + 1) * P, jc * FD:(jc + 1) * FD],
                            in_=gev[:],
                        )

            # ---- phase 3b: pack scalar partials, cross-partition reduce -----
            nc.vector.tensor_copy(sc_pack[:, 0:1], acc_A[:])
            nc.vector.tensor_copy(sc_pack[:, 1:2], acc_B[:])
            nc.vector.tensor_copy(sc_pack[:, 2:3], acc_C[:])
            sc_ps = psum_sc_pool.tile([1, 4], F32, name="sc_ps", tag="sc")
            nc.tensor.matmul(sc_ps[:], ones[:], sc_pack[:], start=True, stop=True)
            sc_sb = evict_pool.tile([1, 8], F32, name="sc_sb", tag="sc_sb")
            nc.vector.memset(sc_sb[:], 0.0)
            nc.vector.tensor_copy(sc_sb[0:1, 0:4], sc_ps[:])
            nc.sync.dma_start(out=sc_part[:], in_=sc_sb[:])

            # ---- phase 4: collectives (overlap with y matmuls below) --------
            nc.gpsimd.collective_compute(
                "ReduceScatter",
                mybir.AluOpType.add,
                replica_groups=RG,
                ins=[g_part[:]],
                outs=[g_red[:]],
            )
            nc.gpsimd.collective_compute(
                "AllReduce",
                mybir.AluOpType.add,
                replica_groups=RG,
                ins=[sc_part[:]],
                outs=[sc_red[:]],
            )

            # bias/halfeye loads are only needed for evictions / the tail
            nc.sync.dma_start(out=bias_sb[:], in_=bias_b[:])
            nc.sync.dma_start(out=he_sb[:], in_=halfeye[:])

            # ---- phase 5: y = x @ Wd^T + bias (the big one) -----------------
            xT_r = xT.rearrange("(kt p) t -> p kt t", p=P)
            for sc in range(N_SC):
                slab = xt_pool.tile([P, KT, SC_TOK], F32R, name="slab", tag="slab")
                nc.sync.dma_start(
                    out=slab[:], in_=xT_r[:, :, sc * SC_TOK:(sc + 1) * SC_TOK]
                )
                for t in range(T_PER_SC):
                    for jc in range(OC):
                        ps = psum_pool.tile([P, FD], F32, name="mm_ps", tag="mm")
                        for k in range(KT):
                            nc.tensor.matmul(
                                ps[:],
                                slab[:, k, t * P:(t + 1) * P],
                                wdT_sb[k][:, jc * FD:(jc + 1) * FD],
                                start=(k == 0),
                                stop=(k == KT - 1),
                            )
                        y_sb = y_pool.tile([P, FD], F32, name="y_sb", tag="y_sb")
                        nc.vector.tensor_add(
                            y_sb[:], ps[:], bias_sb[:, jc * FD:(jc + 1) * FD]
                        )
                        row0 = sc * SC_TOK + t * P
                        nc.scalar.dma_start(
                            out=y_out[row0:row0 + P, jc * FD:(jc + 1) * FD],
                            in_=y_sb[:],
                        )

            # ---- phase 6: grad strip + losses (after collectives) -----------
            g_strip = res.tile([P, D], F32, name="g_strip")
            grad_sb = g_strip
            nc.sync.dma_start(out=g_strip[:], in_=g_red[:])
            nc.vector.scalar_tensor_tensor(
                out=grad_sb[:],
                in0=g_strip[:],
                scalar=float((1.0 - KAPPA) / N_ROWS),
                in1=he_sb[:],
                op0=mybir.AluOpType.mult,
                op1=mybir.AluOpType.subtract,
            )
            nc.sync.dma_start(out=grad_out[:], in_=grad_sb[:])

            scr = res.tile([1, 8], F32, name="scr")
            lt = res.tile([1, 8], F32, name="lt")
            losses_sb = res.tile([1, 8], F32, name="losses_sb")
            nc.sync.dma_start(out=scr[:], in_=sc_red[:])
            nc.vector.memset(losses_sb[:], 0.0)
            # corr = (A - B) / DENOM
            nc.vector.tensor_sub(lt[0:1, 0:1], scr[0:1, 0:1], scr[0:1, 1:2])
            nc.scalar.mul(losses_sb[0:1, 0:1], lt[0:1, 0:1], float(1.0 / DENOM))
            # whit = (B - 2C + N*D) / DENOM
            nc.vector.scalar_tensor_tensor(
                out=lt[0:1, 1:2],
                in0=scr[0:1, 2:3],
                scalar=-2.0,
                in1=scr[0:1, 1:2],
                op0=mybir.AluOpType.mult,
                op1=mybir.AluOpType.add,
            )
            nd_tile = res.tile([1, 1], F32, name="nd_tile")
            nc.vector.memset(nd_tile[:], float(N_ROWS * D))
            nc.vector.tensor_add(lt[0:1, 1:2], lt[0:1, 1:2], nd_tile[:])
            nc.scalar.mul(losses_sb[0:1, 1:2], lt[0:1, 1:2], float(1.0 / DENOM))
            nc.sync.dma_start(out=losses_out[:], in_=losses_sb[:])

    nc.compile()
    return nc


_NC_CACHE = None


def _get_nc():
    global _NC_CACHE
    if _NC_CACHE is None:
        _NC_CACHE = _build()
    return _NC_CACHE


def _make_in_maps(x, W, bias, decorr, sample_idx):
    x = np.asarray(x, dtype=np.float32)
    W = np.asarray(W, dtype=np.float32)
    bias = np.asarray(bias, dtype=np.float32)
    decorr = np.asarray(decorr, dtype=np.float32)
    idx = np.asarray(sample_idx).astype(np.int64)

    x2d = x.reshape(TOK, D)
    sel = np.take_along_axis(x, idx[:, :, None], axis=1).reshape(N_ROWS, D)

    wT = np.ascontiguousarray(W.T)
    decorrT = np.ascontiguousarray(decorr.T)
    bias_b = np.ascontiguousarray(np.broadcast_to(bias.reshape(1, D), (P, D)))

    in_maps = []
    for c in range(NCORES):
        he = np.zeros((P, D), dtype=np.float32)
        he[np.arange(P), c * P + np.arange(P)] = KAPPA
        in_maps.append(
            {
                "xT": np.ascontiguousarray(x2d[c * TOK_C:(c + 1) * TOK_C].T),
                "selT": np.ascontiguousarray(sel[c * ROWS_C:(c + 1) * ROWS_C].T),
                "wT": wT,
                "dec_strip": np.ascontiguousarray(decorr[:, c * P:(c + 1) * P]),
                "decorrT": decorrT,
                "bias_b": bias_b,
                "halfeye": he,
            }
        )
    return in_maps


def run(x, W, bias, decorr, sample_idx, trace=False, **trace_kwargs):
    """Run on the 8 NeuronCores; returns ((y, grad, corr, whit), BassKernelResults)."""
    nc = _get_nc()
    in_maps = _make_in_maps(x, W, bias, decorr, sample_idx)
    res = run_bass_kernel_spmd(
        nc, in_maps, core_ids=list(range(NCORES)), trace=trace, **trace_kwargs
    )
    y = np.concatenate([res.results[c]["y"] for c in range(NCORES)], axis=0)
    y = y.reshape(B, L, D)
    grad = np.concatenate([res.results[c]["grad"] for c in range(NCORES)], axis=0)
    losses = res.results[0]["losses"]
    corr = np.float32(losses[0, 0])
    whit = np.float32(losses[0, 1])
    return (y, grad, corr, whit), res


def kernel(x, W, bias, decorr, sample_idx):
    out, _ = run(x, W, bias, decorr, sample_idx, trace=False)
    return out


# revision 18
# speedup vs baseline: 1.0247x; 1.0247x over previous
"""DecorrLinear fused kernel for one TRN2 chip (8 NeuronCores), via Bass/Tile.

Computes, for x:(B,L,D) f32, W:(D,D), bias:(D,), decorr:(D,D), sample_idx:(B,NS):
  y    = x @ (W @ decorr)^T + bias
  xs   = gather(x, sample_idx) @ decorr^T            # (N, D), N = B*NS
  corr = (sum_n r_n^2 - sum X2^2) / (N*D*D)          # r_n = rowsum(X2), X2 = xs^2
  whit = (sum X2^2 - 2*sum X2 + N*D) / (N*D*D)
  grad = 0.5*(xs^T xs / N) - 0.5*I                   # KAPPA = 0.5

Sharding: data-parallel over tokens (y path) and over sampled rows (loss path).
WdT = (W@decorr)^T is computed sharded (each core one 128-row strip) and
AllGathered; G = xs^T xs partials are ReduceScattered so each core finalizes its
own 128-row strip of grad; the three scalar partial sums are AllReduced.

Matmuls run as float32r (fp32 storage, ~tf32 precision, full PE rate at N>=256).
"""

from contextlib import ExitStack

import numpy as np

import concourse.bacc as bacc
import concourse.bass as bass
import concourse.tile as tile
from concourse import mybir
from concourse.bass_utils import run_bass_kernel_spmd

# Problem shape (hardcoded per contract)
B, L, D = 4, 8192, 1024
NS = 2048                      # n_samples per batch row
N_ROWS = B * NS                # 8192 sampled rows total
NCORES = 8
TOK = B * L                    # 32768 tokens total
TOK_C = TOK // NCORES          # 4096 tokens per core
ROWS_C = N_ROWS // NCORES      # 1024 sampled rows per core
KAPPA = 0.5
DENOM = float(N_ROWS) * D * D

F32 = mybir.dt.float32
F32R = mybir.dt.float32r

P = 128                        # partitions
KT = D // P                    # 8 contraction tiles
FD = 512                       # matmul moving free dim
OC = D // FD                   # 2 output column chunks
SC_TOK = 256                   # token superchunk (DMA slab width)
N_SC = TOK_C // SC_TOK         # 8 superchunks
T_PER_SC = SC_TOK // P         # 4 token tiles per superchunk
RG = [list(range(NCORES))]


def _build():
    nc = bacc.Bacc("TRN2", target_bir_lowering=False, debug=False, num_devices=NCORES)

    xT = nc.dram_tensor("xT", [D, TOK_C], F32R, kind="ExternalInput")
    selT = nc.dram_tensor("selT", [D, ROWS_C], F32R, kind="ExternalInput")
    wT = nc.dram_tensor("wT", [D, D], F32R, kind="ExternalInput")
    dec_strip = nc.dram_tensor("dec_strip", [D, P], F32R, kind="ExternalInput")
    decorrT = nc.dram_tensor("decorrT", [D, D], F32R, kind="ExternalInput")
    bias_b = nc.dram_tensor("bias_b", [P, D], F32, kind="ExternalInput")
    halfeye = nc.dram_tensor("halfeye", [P, D], F32, kind="ExternalInput")

    y_out = nc.dram_tensor("y", [TOK_C, D], F32, kind="ExternalOutput")
    grad_out = nc.dram_tensor("grad", [P, D], F32, kind="ExternalOutput")
    losses_out = nc.dram_tensor("losses", [1, 8], F32, kind="ExternalOutput")

    with tile.TileContext(nc) as tc:
        with (
            tc.tile_pool(name="res", bufs=1) as res,          # long-lived SBUF
            tc.tile_pool(name="psum", bufs=6, space="PSUM") as psum_pool,
            tc.tile_pool(name="psum_sc", bufs=1, space="PSUM") as psum_sc_pool,
            tc.tile_pool(name="dram", bufs=1, space="DRAM") as dram,
            tc.tile_pool(name="evict", bufs=2) as evict_pool, # PSUM->SBUF staging
            tc.tile_pool(name="scratch", bufs=1) as scratch,  # stats scratch
            tc.tile_pool(name="xt", bufs=4) as xt_pool,       # y-path slabs
            tc.tile_pool(name="yev", bufs=8) as y_pool,
            tc.tile_pool(name="dctp", bufs=1) as dct_pool,    # decorrT resident
            tc.tile_pool(name="sslabp", bufs=3) as sslab_pool,
        ):
            # ---- long-lived tiles -------------------------------------------
            wdT_sb = [res.tile([P, D], F32R, name=f"wdT{i}") for i in range(KT)]
            bias_sb = res.tile([P, D], F32, name="bias_sb")
            he_sb = res.tile([P, D], F32, name="he_sb")
            acc_A = res.tile([P, 1], F32, name="acc_A")   # sum r_n^2
            acc_B = res.tile([P, 1], F32, name="acc_B")   # sum X2^2
            acc_C = res.tile([P, 1], F32, name="acc_C")   # sum X2
            ones = res.tile([P, 1], F32, name="ones")
            sc_pack = res.tile([P, 4], F32, name="sc_pack")

            nc.vector.memset(acc_A[:], 0.0)
            nc.vector.memset(acc_B[:], 0.0)
            nc.vector.memset(acc_C[:], 0.0)
            nc.vector.memset(ones[:], 1.0)
            nc.vector.memset(sc_pack[:], 0.0)

            g_part = dram.tile([D, D], F32, name="g_part")
            g_red = dram.tile([P, D], F32, name="g_red")
            sc_part = dram.tile([1, 8], F32, name="sc_part")
            sc_red = dram.tile([1, 8], F32, name="sc_red", addr_space="Shared")
            wd_strip_d = dram.tile([P, D], F32, name="wd_strip_d")
            wd_full = dram.tile([D, D], F32, name="wd_full", addr_space="Shared")

            dct_sb = [dct_pool.tile([P, D], F32R, name=f"dct{k}") for k in range(KT)]
            for k in range(KT):
                nc.sync.dma_start(out=dct_sb[k][:], in_=decorrT[k * P:(k + 1) * P, :])

            # ---- phase 1: own 128-row strip of WdT = decorr^T @ W^T ---------
            # core c computes WdT[128c:128c+128, :] = dec_strip^T @ W^T
            with tc.tile_pool(name="w1", bufs=1) as w1:
                ds_sb = [w1.tile([P, P], F32R, name=f"ds{k}") for k in range(KT)]
                wt_sb = [w1.tile([P, D], F32R, name=f"wt{k}") for k in range(KT)]
                for k in range(KT):
                    nc.sync.dma_start(out=ds_sb[k][:], in_=dec_strip[k * P:(k + 1) * P, :])
                    nc.sync.dma_start(out=wt_sb[k][:], in_=wT[k * P:(k + 1) * P, :])
                wds_sb = w1.tile([P, D], F32, name="wds_sb")
                for jc in range(OC):
                    ps = psum_pool.tile([P, FD], F32, name="mm_ps", tag="mm")
                    for k in range(KT):
                        nc.tensor.matmul(
                            ps[:],
                            ds_sb[k][:],
                            wt_sb[k][:, jc * FD:(jc + 1) * FD],
                            start=(k == 0),
                            stop=(k == KT - 1),
                        )
                    nc.vector.tensor_copy(wds_sb[:, jc * FD:(jc + 1) * FD], ps[:])
                nc.sync.dma_start(out=wd_strip_d[:], in_=wds_sb[:])

            nc.gpsimd.collective_compute(
                "AllGather",
                mybir.AluOpType.bypass,
                replica_groups=RG,
                ins=[wd_strip_d[:]],
                outs=[wd_full[:]],
            )
            for k in range(KT):
                nc.gpsimd.dma_start(
                    out=wdT_sb[k][:], in_=wd_full[k * P:(k + 1) * P, :].bitcast(F32R)
                )

            # ---- phase 2: xs = selT^T @ decorr^T + stats --------------------
            with tc.tile_pool(name="w2", bufs=1) as w2:
                xs_sb = [w2.tile([P, D], F32R, name=f"xs{i}") for i in range(KT)]
                selT_r = selT.rearrange("(kt p) r -> p kt r", p=P)
                for rt in range(KT):
                    sslab = sslab_pool.tile([P, KT, P], F32R, name="sslab", tag="sslab")
                    nc.sync.dma_start(
                        out=sslab[:], in_=selT_r[:, :, rt * P:(rt + 1) * P]
                    )
                    for jc in range(OC):
                        ps = psum_pool.tile([P, FD], F32, name="mm_ps", tag="mm")
                        for k in range(KT):
                            nc.tensor.matmul(
                                ps[:],
                                sslab[:, k, :],
                                dct_sb[k][:, jc * FD:(jc + 1) * FD],
                                start=(k == 0),
                                stop=(k == KT - 1),
                            )
                        nc.vector.tensor_copy(xs_sb[rt][:, jc * FD:(jc + 1) * FD], ps[:])

                    # per-row-tile stats on X2 = xs^2
                    xsf = xs_sb[rt][:].bitcast(F32)
                    x2 = scratch.tile([P, D], F32, name="x2", tag="x2")
                    rsum = scratch.tile([P, 1], F32, name="rsum", tag="rsum")
                    tvec = scratch.tile([P, 1], F32, name="tvec", tag="tvec")
                    nc.vector.tensor_mul(x2[:], xsf, xsf)
                    nc.vector.tensor_reduce(
                        rsum[:], x2[:], axis=mybir.AxisListType.X, op=mybir.AluOpType.add
                    )
                    nc.vector.tensor_mul(x2[:], x2[:], x2[:])
                    nc.vector.tensor_reduce(
                        tvec[:], x2[:], axis=mybir.AxisListType.X,
                        op=mybir.AluOpType.add,
                    )
                    nc.vector.scalar_tensor_tensor(
                        out=acc_A[:],
                        in0=rsum[:],
                        scalar=rsum[:, 0:1],
                        in1=acc_A[:],
                        op0=mybir.AluOpType.mult,
                        op1=mybir.AluOpType.add,
                    )
                    nc.vector.tensor_add(acc_C[:], acc_C[:], rsum[:])
                    nc.vector.tensor_add(acc_B[:], acc_B[:], tvec[:])

                # ---- phase 3: G = xs^T @ xs -> DRAM partial -----------------
                for d1 in range(KT):
                    for jc in range(OC):
                        ps = psum_pool.tile([P, FD], F32, name="mm_ps", tag="mm")
                        for rt in range(KT):
                            nc.tensor.matmul(
                                ps[:],
                                xs_sb[rt][:, d1 * P:(d1 + 1) * P],
                                xs_sb[rt][:, jc * FD:(jc + 1) * FD],
                                start=(rt == 0),
                                stop=(rt == KT - 1),
                            )
                        gev = evict_pool.tile([P, FD], F32, name="gev", tag="gev")
                        nc.vector.tensor_copy(gev[:], ps[:])
                        nc.sync.dma_start(
                            out=g_part[d1 * P:(d1 + 1) * P, jc * FD:(jc + 1) * FD],
                            in_=gev[:],
                        )

            # ---- phase 3b: pack scalar partials, cross-partition reduce -----
            nc.vector.tensor_copy(sc_pack[:, 0:1], acc_A[:])
            nc.vector.tensor_copy(sc_pack[:, 1:2], acc_B[:])
            nc.vector.tensor_copy(sc_pack[:, 2:3], acc_C[:])
            sc_ps = psum_sc_pool.tile([1, 4], F32, name="sc_ps", tag="sc")
            nc.tensor.matmul(sc_ps[:], ones[:], sc_pack[:], start=True, stop=True)
            sc_sb = evict_pool.tile([1, 8], F32, name="sc_sb", tag="sc_sb")
            nc.vector.memset(sc_sb[:], 0.0)
            nc.vector.tensor_copy(sc_sb[0:1, 0:4], sc_ps[:])
            nc.sync.dma_start(out=sc_part[:], in_=sc_sb[:])

            # ---- phase 4: collectives (overlap with y matmuls below) --------
            nc.gpsimd.collective_compute(
                "ReduceScatter",
                mybir.AluOpType.add,
                replica_groups=RG,
                ins=[g_part[:]],
                outs=[g_red[:]],
            )
            nc.gpsimd.collective_compute(
                "AllReduce",
                mybir.AluOpType.add,
                replica_groups=RG,
                ins=[sc_part[:]],
                outs=[sc_red[:]],
            )

            # bias/halfeye loads are only needed for evictions / the tail
            nc.sync.dma_start(out=bias_sb[:], in_=bias_b[:])
            nc.sync.dma_start(out=he_sb[:], in_=halfeye[:])

            # ---- phase 5: y = x @ Wd^T + bias (the big one) -----------------
            xT_r = xT.rearrange("(kt p) t -> p kt t", p=P)
            for sc in range(N_SC):
                slab = xt_pool.tile([P, KT, SC_TOK], F32R, name="slab", tag="slab")
                nc.sync.dma_start(
                    out=slab[:], in_=xT_r[:, :, sc * SC_TOK:(sc + 1) * SC_TOK]
                )
                for t in range(T_PER_SC):
                    for jc in range(OC):
                        ps = psum_pool.tile([P, FD], F32, name="mm_ps", tag="mm")
                        for k in range(KT):
                            nc.tensor.matmul(
                                ps[:],
                                slab[:, k, t * P:(t + 1) * P],
                                wdT_sb[k][:, jc * FD:(jc + 1) * FD],
                                start=(k == 0),
                                stop=(k == KT - 1),
                            )
                        y_sb = y_pool.tile([P, FD], F32, name="y_sb", tag="y_sb")
                        nc.vector.tensor_add(
                            y_sb[:], ps[:], bias_sb[:, jc * FD:(jc + 1) * FD]
                        )
                        row0 = sc * SC_TOK + t * P
                        nc.sync.dma_start(
                            out=y_out[row0:row0 + P, jc * FD:(jc + 1) * FD],
                            in_=y_sb[:],
                        )

            # ---- phase 6: grad strip + losses (after collectives) -----------
            g_strip = res.tile([P, D], F32, name="g_strip")
            grad_sb = g_strip
            nc.sync.dma_start(out=g_strip[:], in_=g_red[:])
            nc.vector.scalar_tensor_tensor(
                out=grad_sb[:],
                in0=g_strip[:],
                scalar=float((1.0 - KAPPA) / N_ROWS),
                in1=he_sb[:],
                op0=mybir.AluOpType.mult,
                op1=mybir.AluOpType.subtract,
            )
            nc.sync.dma_start(out=grad_out[:], in_=grad_sb[:])

            scr = res.tile([1, 8], F32, name="scr")
            lt = res.tile([1, 8], F32, name="lt")
            losses_sb = res.tile([1, 8], F32, name="losses_sb")
            nc.sync.dma_start(out=scr[:], in_=sc_red[:])
            nc.vector.memset(losses_sb[:], 0.0)
            # corr = (A - B) / DENOM
            nc.vector.tensor_sub(lt[0:1, 0:1], scr[0:1, 0:1], scr[0:1, 1:2])
            nc.scalar.mul(losses_sb[0:1, 0:1], lt[0:1, 0:1], float(1.0 / DENOM))
            # whit = (B - 2C + N*D) / DENOM
            nc.vector.scalar_tensor_tensor(
                out=lt[0:1, 1:2],
                in0=scr[0:1, 2:3],
                scalar=-2.0,
                in1=scr[0:1, 1:2],
                op0=mybir.AluOpType.mult,
                op1=mybir.AluOpType.add,
            )
            nd_tile = res.tile([1, 1], F32, name="nd_tile")
            nc.vector.memset(nd_tile[:], float(N_ROWS * D))
            nc.vector.tensor_add(lt[0:1, 1:2], lt[0:1, 1:2], nd_tile[:])
            nc.scalar.mul(losses_sb[0:1, 1:2], lt[0:1, 1:2], float(1.0 / DENOM))
            nc.sync.dma_start(out=losses_out[:], in_=losses_sb[:])

    nc.compile()
    return nc


_NC_CACHE = None


def _get_nc():
    global _NC_CACHE
    if _NC_CACHE is None:
        _NC_CACHE = _build()
    return _NC_CACHE


def _make_in_maps(x, W, bias, decorr, sample_idx):
    x = np.asarray(x, dtype=np.float32)
    W = np.asarray(W, dtype=np.float32)
    bias = np.asarray(bias, dtype=np.float32)
    decorr = np.asarray(decorr, dtype=np.float32)
    idx = np.asarray(sample_idx).astype(np.int64)

    x2d = x.reshape(TOK, D)
    sel = np.take_along_axis(x, idx[:, :, None], axis=1).reshape(N_ROWS, D)

    wT = np.ascontiguousarray(W.T)
    decorrT = np.ascontiguousarray(decorr.T)
    bias_b = np.ascontiguousarray(np.broadcast_to(bias.reshape(1, D), (P, D)))

    in_maps = []
    for c in range(NCORES):
        he = np.zeros((P, D), dtype=np.float32)
        he[np.arange(P), c * P + np.arange(P)] = KAPPA
        in_maps.append(
            {
                "xT": np.ascontiguousarray(x2d[c * TOK_C:(c + 1) * TOK_C].T),
                "selT": np.ascontiguousarray(sel[c * ROWS_C:(c + 1) * ROWS_C].T),
                "wT": wT,
                "dec_strip": np.ascontiguousarray(decorr[:, c * P:(c + 1) * P]),
                "decorrT": decorrT,
                "bias_b": bias_b,
                "halfeye": he,
            }
        )
    return in_maps


def run(x, W, bias, decorr, sample_idx, trace=False, **trace_kwargs):
    """Run on the 8 NeuronCores; returns ((y, grad, corr, whit), BassKernelResults)."""
    nc = _get_nc()
    in_maps = _make_in_maps(x, W, bias, decorr, sample_idx)
    res = run_bass_kernel_spmd(
        nc, in_maps, core_ids=list(range(NCORES)), trace=trace, **trace_kwargs
    )
    y = np.concatenate([res.results[c]["y"] for c in range(NCORES)], axis=0)
    y = y.reshape(B, L, D)
    grad = np.concatenate([res.results[c]["grad"] for c in range(NCORES)], axis=0)
    losses = res.results[0]["losses"]
    corr = np.float32(losses[0, 0])
    whit = np.float32(losses[0, 1])
    return (y, grad, corr, whit), res


def kernel(x, W, bias, decorr, sample_idx):
    out, _ = run(x, W, bias, decorr, sample_idx, trace=False)
    return out


# revision 19
# speedup vs baseline: 1.0677x; 1.0420x over previous
"""DecorrLinear fused kernel for one TRN2 chip (8 NeuronCores), via Bass/Tile.

Computes, for x:(B,L,D) f32, W:(D,D), bias:(D,), decorr:(D,D), sample_idx:(B,NS):
  y    = x @ (W @ decorr)^T + bias
  xs   = gather(x, sample_idx) @ decorr^T            # (N, D), N = B*NS
  corr = (sum_n r_n^2 - sum X2^2) / (N*D*D)          # r_n = rowsum(X2), X2 = xs^2
  whit = (sum X2^2 - 2*sum X2 + N*D) / (N*D*D)
  grad = 0.5*(xs^T xs / N) - 0.5*I                   # KAPPA = 0.5

Sharding: data-parallel over tokens (y path) and over sampled rows (loss path).
WdT = (W@decorr)^T is computed sharded (each core one 128-row strip) and
AllGathered; G = xs^T xs partials are ReduceScattered so each core finalizes its
own 128-row strip of grad; the three scalar partial sums are AllReduced.

Matmuls run as float32r (fp32 storage, ~tf32 precision, full PE rate at N>=256).
"""

from contextlib import ExitStack

import numpy as np

import concourse.bacc as bacc
import concourse.bass as bass
import concourse.tile as tile
from concourse import mybir
from concourse.bass_utils import run_bass_kernel_spmd

# Problem shape (hardcoded per contract)
B, L, D = 4, 8192, 1024
NS = 2048                      # n_samples per batch row
N_ROWS = B * NS                # 8192 sampled rows total
NCORES = 8
TOK = B * L                    # 32768 tokens total
TOK_C = TOK // NCORES          # 4096 tokens per core
ROWS_C = N_ROWS // NCORES      # 1024 sampled rows per core
KAPPA = 0.5
DENOM = float(N_ROWS) * D * D

F32 = mybir.dt.float32
F32R = mybir.dt.float32r

P = 128                        # partitions
KT = D // P                    # 8 contraction tiles
FD = 512                       # matmul moving free dim
OC = D // FD                   # 2 output column chunks
SC_TOK = 256                   # token superchunk (DMA slab width)
N_SC = TOK_C // SC_TOK         # 8 superchunks
T_PER_SC = SC_TOK // P         # 4 token tiles per superchunk
RG = [list(range(NCORES))]


def _build():
    nc = bacc.Bacc("TRN2", target_bir_lowering=False, debug=False, num_devices=NCORES)

    xT = nc.dram_tensor("xT", [D, TOK_C], F32R, kind="ExternalInput")
    selT = nc.dram_tensor("selT", [D, ROWS_C], F32R, kind="ExternalInput")
    wT = nc.dram_tensor("wT", [D, D], F32R, kind="ExternalInput")
    dec_strip = nc.dram_tensor("dec_strip", [D, P], F32R, kind="ExternalInput")
    decorrT = nc.dram_tensor("decorrT", [D, D], F32R, kind="ExternalInput")
    bias_b = nc.dram_tensor("bias_b", [P, D], F32, kind="ExternalInput")
    halfeye = nc.dram_tensor("halfeye", [P, D], F32, kind="ExternalInput")

    y_out = nc.dram_tensor("y", [TOK_C, D], F32, kind="ExternalOutput")
    grad_out = nc.dram_tensor("grad", [P, D], F32, kind="ExternalOutput")
    losses_out = nc.dram_tensor("losses", [1, 8], F32, kind="ExternalOutput")

    with tile.TileContext(nc) as tc:
        with (
            tc.tile_pool(name="res", bufs=1) as res,          # long-lived SBUF
            tc.tile_pool(name="psum", bufs=7, space="PSUM") as psum_pool,
            tc.tile_pool(name="psum_sc", bufs=1, space="PSUM") as psum_sc_pool,
            tc.tile_pool(name="dram", bufs=1, space="DRAM") as dram,
            tc.tile_pool(name="evict", bufs=3) as evict_pool, # PSUM->SBUF staging
            tc.tile_pool(name="scratch", bufs=1) as scratch,  # stats scratch
            tc.tile_pool(name="xt", bufs=4) as xt_pool,       # y-path slabs
            tc.tile_pool(name="yev", bufs=8) as y_pool,
            tc.tile_pool(name="dctp", bufs=1) as dct_pool,    # decorrT resident
            tc.tile_pool(name="sslabp", bufs=3) as sslab_pool,
        ):
            # ---- long-lived tiles -------------------------------------------
            wdT_sb = [res.tile([P, D], F32R, name=f"wdT{i}") for i in range(KT)]
            bias_sb = res.tile([P, D], F32, name="bias_sb")
            he_sb = res.tile([P, D], F32, name="he_sb")
            acc_A = res.tile([P, 1], F32, name="acc_A")   # sum r_n^2
            acc_B = res.tile([P, 1], F32, name="acc_B")   # sum X2^2
            acc_C = res.tile([P, 1], F32, name="acc_C")   # sum X2
            ones = res.tile([P, 1], F32, name="ones")
            sc_pack = res.tile([P, 4], F32, name="sc_pack")

            nc.vector.memset(acc_A[:], 0.0)
            nc.vector.memset(acc_B[:], 0.0)
            nc.vector.memset(acc_C[:], 0.0)
            nc.vector.memset(ones[:], 1.0)
            nc.vector.memset(sc_pack[:], 0.0)

            g_part = dram.tile([D, D], F32, name="g_part")
            g_red = dram.tile([P, D], F32, name="g_red")
            sc_part = dram.tile([1, 8], F32, name="sc_part")
            sc_red = dram.tile([1, 8], F32, name="sc_red", addr_space="Shared")
            wd_strip_d = dram.tile([P, D], F32, name="wd_strip_d")
            wd_full = dram.tile([D, D], F32, name="wd_full", addr_space="Shared")

            dct_sb = [dct_pool.tile([P, D], F32R, name=f"dct{k}") for k in range(KT)]
            for k in range(KT):
                nc.sync.dma_start(out=dct_sb[k][:], in_=decorrT[k * P:(k + 1) * P, :])

            # ---- phase 1: own 128-row strip of WdT = decorr^T @ W^T ---------
            # core c computes WdT[128c:128c+128, :] = dec_strip^T @ W^T
            with tc.tile_pool(name="w1", bufs=1) as w1:
                ds_sb = [w1.tile([P, P], F32R, name=f"ds{k}") for k in range(KT)]
                wt_sb = [w1.tile([P, D], F32R, name=f"wt{k}") for k in range(KT)]
                for k in range(KT):
                    nc.sync.dma_start(out=ds_sb[k][:], in_=dec_strip[k * P:(k + 1) * P, :])
                for jc in range(OC):
                    for k in range(KT):
                        nc.sync.dma_start(
                            out=wt_sb[k][:, jc * FD:(jc + 1) * FD],
                            in_=wT[k * P:(k + 1) * P, jc * FD:(jc + 1) * FD],
                        )
                wds_sb = w1.tile([P, D], F32, name="wds_sb")
                for jc in range(OC):
                    ps = psum_pool.tile([P, FD], F32, name="mm_ps", tag="mm")
                    for k in range(KT):
                        nc.tensor.matmul(
                            ps[:],
                            ds_sb[k][:],
                            wt_sb[k][:, jc * FD:(jc + 1) * FD],
                            start=(k == 0),
                            stop=(k == KT - 1),
                        )
                    nc.vector.tensor_copy(wds_sb[:, jc * FD:(jc + 1) * FD], ps[:])
                nc.sync.dma_start(out=wd_strip_d[:], in_=wds_sb[:])

            nc.gpsimd.collective_compute(
                "AllGather",
                mybir.AluOpType.bypass,
                replica_groups=RG,
                ins=[wd_strip_d[:]],
                outs=[wd_full[:]],
            )
            for k in range(KT):
                nc.gpsimd.dma_start(
                    out=wdT_sb[k][:], in_=wd_full[k * P:(k + 1) * P, :].bitcast(F32R)
                )

            # ---- phase 2: xs = selT^T @ decorr^T + stats --------------------
            with tc.tile_pool(name="w2", bufs=1) as w2:
                xs_sb = [w2.tile([P, D], F32R, name=f"xs{i}") for i in range(KT)]
                selT_r = selT.rearrange("(kt p) r -> p kt r", p=P)
                for rt in range(KT):
                    sslab = sslab_pool.tile([P, KT, P], F32R, name="sslab", tag="sslab")
                    nc.sync.dma_start(
                        out=sslab[:], in_=selT_r[:, :, rt * P:(rt + 1) * P]
                    )
                    for jc in range(OC):
                        ps = psum_pool.tile([P, FD], F32, name="mm_ps", tag="mm")
                        for k in range(KT):
                            nc.tensor.matmul(
                                ps[:],
                                sslab[:, k, :],
                                dct_sb[k][:, jc * FD:(jc + 1) * FD],
                                start=(k == 0),
                                stop=(k == KT - 1),
                            )
                        nc.vector.tensor_copy(xs_sb[rt][:, jc * FD:(jc + 1) * FD], ps[:])

                    # per-row-tile stats on X2 = xs^2
                    xsf = xs_sb[rt][:].bitcast(F32)
                    x2 = scratch.tile([P, D], F32, name="x2", tag="x2")
                    rsum = scratch.tile([P, 1], F32, name="rsum", tag="rsum")
                    tvec = scratch.tile([P, 1], F32, name="tvec", tag="tvec")
                    nc.vector.tensor_mul(x2[:], xsf, xsf)
                    nc.vector.tensor_reduce(
                        rsum[:], x2[:], axis=mybir.AxisListType.X, op=mybir.AluOpType.add
                    )
                    nc.vector.tensor_mul(x2[:], x2[:], x2[:])
                    nc.vector.tensor_reduce(
                        tvec[:], x2[:], axis=mybir.AxisListType.X,
                        op=mybir.AluOpType.add,
                    )
                    nc.vector.scalar_tensor_tensor(
                        out=acc_A[:],
                        in0=rsum[:],
                        scalar=rsum[:, 0:1],
                        in1=acc_A[:],
                        op0=mybir.AluOpType.mult,
                        op1=mybir.AluOpType.add,
                    )
                    nc.vector.tensor_add(acc_C[:], acc_C[:], rsum[:])
                    nc.vector.tensor_add(acc_B[:], acc_B[:], tvec[:])

                # ---- phase 3: G = xs^T @ xs -> DRAM partial -----------------
                for d1 in range(KT):
                    for jc in range(OC):
                        ps = psum_pool.tile([P, FD], F32, name="mm_ps", tag="mm")
                        for rt in range(KT):
                            nc.tensor.matmul(
                                ps[:],
                                xs_sb[rt][:, d1 * P:(d1 + 1) * P],
                                xs_sb[rt][:, jc * FD:(jc + 1) * FD],
                                start=(rt == 0),
                                stop=(rt == KT - 1),
                            )
                        gev = evict_pool.tile([P, FD], F32, name="gev", tag="gev")
                        nc.vector.tensor_copy(gev[:], ps[:])
                        nc.sync.dma_start(
                            out=g_part[d1 * P:(d1 + 1) * P, jc * FD:(jc + 1) * FD],
                            in_=gev[:],
                        )

            # ---- phase 3b: pack scalar partials, cross-partition reduce -----
            nc.vector.tensor_copy(sc_pack[:, 0:1], acc_A[:])
            nc.vector.tensor_copy(sc_pack[:, 1:2], acc_B[:])
            nc.vector.tensor_copy(sc_pack[:, 2:3], acc_C[:])
            sc_ps = psum_sc_pool.tile([1, 4], F32, name="sc_ps", tag="sc")
            nc.tensor.matmul(sc_ps[:], ones[:], sc_pack[:], start=True, stop=True)
            sc_sb = evict_pool.tile([1, 8], F32, name="sc_sb", tag="sc_sb")
            nc.vector.memset(sc_sb[:], 0.0)
            nc.vector.tensor_copy(sc_sb[0:1, 0:4], sc_ps[:])
            nc.sync.dma_start(out=sc_part[:], in_=sc_sb[:])

            # ---- phase 4: collectives (overlap with y matmuls below) --------
            nc.gpsimd.collective_compute(
                "ReduceScatter",
                mybir.AluOpType.add,
                replica_groups=RG,
                ins=[g_part[:]],
                outs=[g_red[:]],
            )
            nc.gpsimd.collective_compute(
                "AllReduce",
                mybir.AluOpType.add,
                replica_groups=RG,
                ins=[sc_part[:]],
                outs=[sc_red[:]],
            )

            # bias/halfeye loads are only needed for evictions / the tail
            nc.sync.dma_start(out=bias_sb[:], in_=bias_b[:])
            nc.sync.dma_start(out=he_sb[:], in_=halfeye[:])

            # ---- phase 5: y = x @ Wd^T + bias (the big one) -----------------
            xT_r = xT.rearrange("(kt p) t -> p kt t", p=P)
            for sc in range(N_SC):
                slab = xt_pool.tile([P, KT, SC_TOK], F32R, name="slab", tag="slab")
                nc.sync.dma_start(
                    out=slab[:], in_=xT_r[:, :, sc * SC_TOK:(sc + 1) * SC_TOK]
                )
                for t in range(T_PER_SC):
                    for jc in range(OC):
                        ps = psum_pool.tile([P, FD], F32, name="mm_ps", tag="mm")
                        for k in range(KT):
                            nc.tensor.matmul(
                                ps[:],
                                slab[:, k, t * P:(t + 1) * P],
                                wdT_sb[k][:, jc * FD:(jc + 1) * FD],
                                start=(k == 0),
                                stop=(k == KT - 1),
                            )
                        y_sb = y_pool.tile([P, FD], F32, name="y_sb", tag="y_sb")
                        nc.vector.tensor_add(
                            y_sb[:], ps[:], bias_sb[:, jc * FD:(jc + 1) * FD]
                        )
                        row0 = sc * SC_TOK + t * P
                        nc.sync.dma_start(
                            out=y_out[row0:row0 + P, jc * FD:(jc + 1) * FD],
                            in_=y_sb[:],
                        )

            # ---- phase 6: grad strip + losses (after collectives) -----------
            g_strip = res.tile([P, D], F32, name="g_strip")
            grad_sb = g_strip
            nc.sync.dma_start(out=g_strip[:], in_=g_red[:])
            nc.vector.scalar_tensor_tensor(
                out=grad_sb[:],
                in0=g_strip[:],
                scalar=float((1.0 - KAPPA) / N_ROWS),
                in1=he_sb[:],
                op0=mybir.AluOpType.mult,
                op1=mybir.AluOpType.subtract,
            )
            nc.sync.dma_start(out=grad_out[:], in_=grad_sb[:])

            scr = res.tile([1, 8], F32, name="scr")
            lt = res.tile([1, 8], F32, name="lt")
            losses_sb = res.tile([1, 8], F32, name="losses_sb")
            nc.sync.dma_start(out=scr[:], in_=sc_red[:])
            nc.vector.memset(losses_sb[:], 0.0)
            # corr = (A - B) / DENOM
            nc.vector.tensor_sub(lt[0:1, 0:1], scr[0:1, 0:1], scr[0:1, 1:2])
            nc.scalar.mul(losses_sb[0:1, 0:1], lt[0:1, 0:1], float(1.0 / DENOM))
            # whit = (B - 2C + N*D) / DENOM
            nc.vector.scalar_tensor_tensor(
                out=lt[0:1, 1:2],
                in0=scr[0:1, 2:3],
                scalar=-2.0,
                in1=scr[0:1, 1:2],
                op0=mybir.AluOpType.mult,
                op1=mybir.AluOpType.add,
            )
            nd_tile = res.tile([1, 1], F32, name="nd_tile")
            nc.vector.memset(nd_tile[:], float(N_ROWS * D))
            nc.vector.tensor_add(lt[0:1, 1:2], lt[0:1, 1:2], nd_tile[:])
            nc.scalar.mul(losses_sb[0:1, 1:2], lt[0:1, 1:2], float(1.0 / DENOM))
            nc.sync.dma_start(out=losses_out[:], in_=losses_sb[:])

    nc.compile()
    return nc


_NC_CACHE = None


def _get_nc():
    global _NC_CACHE
    if _NC_CACHE is None:
        _NC_CACHE = _build()
    return _NC_CACHE


def _make_in_maps(x, W, bias, decorr, sample_idx):
    x = np.asarray(x, dtype=np.float32)
    W = np.asarray(W, dtype=np.float32)
    bias = np.asarray(bias, dtype=np.float32)
    decorr = np.asarray(decorr, dtype=np.float32)
    idx = np.asarray(sample_idx).astype(np.int64)

    x2d = x.reshape(TOK, D)
    sel = np.take_along_axis(x, idx[:, :, None], axis=1).reshape(N_ROWS, D)

    wT = np.ascontiguousarray(W.T)
    decorrT = np.ascontiguousarray(decorr.T)
    bias_b = np.ascontiguousarray(np.broadcast_to(bias.reshape(1, D), (P, D)))

    in_maps = []
    for c in range(NCORES):
        he = np.zeros((P, D), dtype=np.float32)
        he[np.arange(P), c * P + np.arange(P)] = KAPPA
        in_maps.append(
            {
                "xT": np.ascontiguousarray(x2d[c * TOK_C:(c + 1) * TOK_C].T),
                "selT": np.ascontiguousarray(sel[c * ROWS_C:(c + 1) * ROWS_C].T),
                "wT": wT,
                "dec_strip": np.ascontiguousarray(decorr[:, c * P:(c + 1) * P]),
                "decorrT": decorrT,
                "bias_b": bias_b,
                "halfeye": he,
            }
        )
    return in_maps


def run(x, W, bias, decorr, sample_idx, trace=False, **trace_kwargs):
    """Run on the 8 NeuronCores; returns ((y, grad, corr, whit), BassKernelResults)."""
    nc = _get_nc()
    in_maps = _make_in_maps(x, W, bias, decorr, sample_idx)
    res = run_bass_kernel_spmd(
        nc, in_maps, core_ids=list(range(NCORES)), trace=trace, **trace_kwargs
    )
    y = np.concatenate([res.results[c]["y"] for c in range(NCORES)], axis=0)
    y = y.reshape(B, L, D)
    grad = np.concatenate([res.results[c]["grad"] for c in range(NCORES)], axis=0)
    losses = res.results[0]["losses"]
    corr = np.float32(losses[0, 0])
    whit = np.float32(losses[0, 1])
    return (y, grad, corr, whit), res


def kernel(x, W, bias, decorr, sample_idx):
    out, _ = run(x, W, bias, decorr, sample_idx, trace=False)
    return out
